# revision 17
# baseline (speedup 1.0000x reference)
"""Multi-head attention (b=2, n=2048, dim=1024, 16 heads x 64) on 8 TRN2 NeuronCores.

Sharding: core c handles batch c//4 and heads 4*(c%4) .. 4*(c%4)+3
(data parallel over batch x 4-way head/tensor parallel). w_qkv is
column-sharded by head; w_out is column-sharded: each core computes a
256-column slice of the output after per-block AllGathers of the
attention outputs within its 4-core batch group.

v5: the attention inner loop is paced by the softmax exp. The exp work is
split between the scalar engine (exact, activation table) and the vector
engine (Schraudolph: bf16(e^x) bits == int16(x*128*log2e/sqrt(d) + 16250),
one fused tensor_scalar op, ~2% error on 4/16 of the key chunks). All
Q/K/V projections are interleaved into the pipeline; the first
query-block's Q/K and two V tiles run as four interleaved accumulation
chains that stream behind the input DMA. Per-block AllGathers fire as
each block's normalized output is ready, and per-block output-projection
passes are injected 1.5 blocks later. Input DMAs alternate between the
SP and GPSIMD DGE queues.
"""

import sys

sys.path.insert(0, "/opt/trn_rl_repo")

import ml_dtypes
import numpy as np

import concourse.bass as bass  # noqa: F401  (engine types)
import concourse.tile as tile
from concourse import bacc, mybir
from concourse.bass_utils import run_bass_kernel_spmd

F32 = mybir.dt.float32
BF16 = mybir.dt.bfloat16
I16 = mybir.dt.int16
NP_BF16 = np.dtype(ml_dtypes.bfloat16)

# Problem constants
B, N, DIM = 2, 2048, 1024
HEADS, DH = 16, 64
INNER = HEADS * DH
SCALE = DH ** -0.5
CORES = 8
GROUP_SIZE = 4
REPLICA_GROUPS = [[0, 1, 2, 3], [4, 5, 6, 7]]
HPC = 4  # heads per core
CS = HPC * DH  # 256 per-core feature columns

KC = DIM // 128  # 8 contraction chunks for dim
TT = N // 128  # 16 token tiles
QB = N // 512  # 4 q blocks
NKC = N // 128  # 16 key chunks
NBLK = 2 * QB  # 8 attention blocks (head-pair x query-block)

# Schraudolph exp on DVE for these key chunks (kept off kc 0-4 where the
# DVE runs the previous block's normalization chain)
DVE_KCS = (5, 11)
EXP_A = 128.0 * 1.4426950408889634 * SCALE
EXP_B = 128.0 * 127 - 6.0


def build_nc():
    nc = bacc.Bacc("TRN2", target_bir_lowering=False, debug=False, num_devices=CORES)
    xt = nc.dram_tensor("xt", [DIM, N], BF16, kind="ExternalInput").ap()
    wq = nc.dram_tensor("wq", [DIM, CS], BF16, kind="ExternalInput").ap()
    wk = nc.dram_tensor("wk", [DIM, CS], BF16, kind="ExternalInput").ap()
    wv = nc.dram_tensor("wv", [DIM, CS], BF16, kind="ExternalInput").ap()
    wo = nc.dram_tensor("wo", [INNER, CS], BF16, kind="ExternalInput").ap()
    bo = nc.dram_tensor("bo", [CS], F32, kind="ExternalInput").ap()
    y = nc.dram_tensor("y", [CS, N], F32, kind="ExternalOutput").ap()  # y^T

    cc_in = [nc.dram_tensor(f"cc_in{b}", [128, 512], BF16) for b in range(NBLK)]
    cc_out = [
        nc.dram_tensor(f"cc_out{b}", [GROUP_SIZE * 128, 512], BF16)
        for b in range(NBLK)
    ]
    cc_in_h = {
        (b, h): nc.dram_tensor(f"cc_inh{b}_{h}", [128, 256], BF16)
        for b in (NBLK - 2, NBLK - 1)
        for h in range(2)
    }
    cc_out_h = {
        (b, h): nc.dram_tensor(f"cc_outh{b}_{h}", [GROUP_SIZE * 128, 256], BF16)
        for b in (NBLK - 2, NBLK - 1)
        for h in range(2)
    }

    with tile.TileContext(nc) as tc:
        with (
            tc.tile_pool(name="big", bufs=2) as big,  # xt, then the AG results
            tc.tile_pool(name="sb", bufs=1) as sb,
            tc.tile_pool(name="expp", bufs=4) as expp,
            tc.tile_pool(name="yout", bufs=3) as yout,
            tc.tile_pool(name="norm", bufs=4) as normp,
            tc.tile_pool(name="psd", bufs=2, space="PSUM") as psd,
            tc.tile_pool(name="pso", bufs=2, space="PSUM") as pso,
            tc.tile_pool(name="psy", bufs=2, space="PSUM") as psyp,
        ):
            # ---- input DMAs -------------------------------------------------
            xt_sb = big.tile([128, KC, N], BF16, tag="bigbuf")
            wq_sb = sb.tile([128, KC, CS], BF16)
            wk_sb = sb.tile([128, KC, CS], BF16)
            wv_sb = sb.tile([128, KC, CS], BF16)
            wo_sb = sb.tile([128, KC, CS], BF16)
            bias_sb = sb.tile([128, 2], F32)
            wq_r = wq.rearrange("(c p) n -> p c n", p=128)
            wk_r = wk.rearrange("(c p) n -> p c n", p=128)
            xt_r = xt.rearrange("(c p) n -> p c n", p=128)
            # priority order; alternate between the SP and GPSIMD DGE rings
            dmas = [
                (wk_sb[:, :, 0:128], wk_r[:, :, 0:128]),
                (wq_sb[:, :, 0:128], wq_r[:, :, 0:128]),
            ]
            for c in range(KC):
                dmas.append((xt_sb[:, c, 0:512], xt_r[:, c, 0:512]))
            dmas.append((wv_sb, wv.rearrange("(c p) n -> p c n", p=128)))
            dmas.append((wk_sb[:, :, 128:256], wk_r[:, :, 128:256]))
            dmas.append((wq_sb[:, :, 128:256], wq_r[:, :, 128:256]))
            for qb in range(1, QB):
                sl = slice(qb * 512, (qb + 1) * 512)
                for c in range(KC):
                    dmas.append((xt_sb[:, c, sl], xt_r[:, c, sl]))
            qs = [nc.sync, nc.gpsimd]
            for i, (dst, srcap) in enumerate(dmas):
                qs[i % 2].dma_start(out=dst, in_=srcap)
            # wo/bias ride the scalar DGE ring: it is idle until the first
            # exp (~24us) and these triggers fire unconditionally at boot
            nc.scalar.dma_start(out=wo_sb, in_=wo.rearrange("(c p) n -> p c n", p=128))
            nc.scalar.dma_start(out=bias_sb, in_=bo.rearrange("(cb p) -> p cb", p=128))

            ones_f = sb.tile([128, TT], F32)
            nc.vector.memset(ones_f, 1.0)
            ones_b = sb.tile([1, DH], BF16)
            with nc.allow_low_precision(reason="bf16 ones"):
                nc.vector.tensor_copy(ones_b, ones_f[0:1, 0:1].broadcast_to([1, DH]))

            qt_sb = sb.tile([128, 2, N], BF16)
            kt_sb = sb.tile([128, 2, N], BF16)
            vaug = sb.tile([128, TT, HPC, DH + 1], BF16)
            with nc.allow_low_precision(reason="bf16 ones column"):
                for h in range(HPC):
                    nc.vector.tensor_copy(vaug[:, :, h, DH], ones_f)

            y_acc = sb.tile([128, 2, N], F32)
            outt_sb = sb.tile([128, 2, N], BF16)
            ag_all = big.tile([128, 2, QB, GROUP_SIZE, 512], BF16, tag="bigbuf")

            # ---- building blocks --------------------------------------------
            def emit_qk(m, qb, dst, w_sb, pool, pname):
                ps = pool.tile([128, 512], F32, name=pname)
                for c in range(KC):
                    nc.tensor.matmul(
                        ps,
                        lhsT=w_sb[:, c, m * 128 : (m + 1) * 128],
                        rhs=xt_sb[:, c, qb * 512 : (qb + 1) * 512],
                        start=(c == 0),
                        stop=(c == KC - 1),
                    )
                with nc.allow_low_precision(reason="bf16 attention"):
                    nc.vector.tensor_copy(dst[:, m, qb * 512 : (qb + 1) * 512], ps)

            def emit_v(t, pool):
                ps = pool.tile([128, 512], F32, name="psy")
                acc = ps[:, 0:CS]
                for c in range(KC):
                    nc.tensor.matmul(
                        acc,
                        lhsT=xt_sb[:, c, t * 128 : (t + 1) * 128],
                        rhs=wv_sb[:, c, :],
                        start=(c == 0),
                        stop=(c == KC - 1),
                    )
                with nc.allow_low_precision(reason="bf16 attention"):
                    nc.vector.tensor_copy(
                        vaug[:, t, :, 0:DH],
                        acc.rearrange("p (h d) -> p h d", d=DH),
                    )

            def emit_dots(blk, kc):
                hp, qb = divmod(blk, QB)
                ps = psd.tile([128, 2, 512], F32, name="psd")
                for hh in range(2):
                    base = hh * DH
                    nc.tensor.matmul(
                        ps[:, hh, :],
                        lhsT=kt_sb[base : base + DH, hp, kc * 128 : (kc + 1) * 128],
                        rhs=qt_sb[base : base + DH, hp, qb * 512 : (qb + 1) * 512],
                        start=True,
                        stop=True,
                        tile_position=(base, 0),
                    )
                if kc in DVE_KCS:
                    # Schraudolph exp on the vector engine: the int16 value
                    # x*A+B is exactly the bf16 bit pattern of e^(x*scale)
                    exi = expp.tile([128, 2, 512], I16, name="expT")
                    with nc.allow_low_precision(reason="schraudolph exp"):
                        nc.vector.tensor_scalar(
                            out=exi,
                            in0=ps,
                            scalar1=EXP_A,
                            scalar2=EXP_B,
                            op0=mybir.AluOpType.mult,
                            op1=mybir.AluOpType.add,
                        )
                    return exi.bitcast(BF16)
                ex = expp.tile([128, 2, 512], BF16, name="expT")
                nc.scalar.activation(
                    out=ex, in_=ps, func=mybir.ActivationFunctionType.Exp, scale=SCALE
                )
                return ex

            def emit_attv(blk, kc, ex, po):
                hp = blk // QB
                for hh in range(2):
                    nc.tensor.matmul(
                        po[hh],
                        lhsT=vaug[:, kc, hp * 2 + hh, :],
                        rhs=ex[:, hh, :],
                        start=(kc == 0),
                        stop=(kc == NKC - 1),
                    )

            def emit_recip(po):
                # one [65,512] copy per head releases the PSUM WAR for the
                # next block's attV as early as possible; the custom-DVE
                # recip needs an SBUF input at base partition 0
                po_sbs, zinvs = [], []
                for hh in range(2):
                    po_sb = normp.tile([DH + 1, 512], F32, name="po_sb")
                    nc.vector.tensor_copy(po_sb, po[hh])
                    zrow = normp.tile([1, 512], F32, name="zrow")
                    nc.vector.tensor_copy(zrow, po_sb[DH : DH + 1, :])
                    zinv = normp.tile([1, 512], F32, name="zinv")
                    nc.vector.reciprocal_approx_fast(zinv, zrow)
                    po_sbs.append(po_sb)
                    zinvs.append(zinv)
                return po_sbs, zinvs

            def emit_norm(blk, po_sbs, zinvs):
                hp, qb = divmod(blk, QB)
                for hh in range(2):
                    base = hh * DH
                    zb = normp.tile([DH, 512], F32, name="zb")
                    nc.gpsimd.partition_broadcast(zb, zinvs[hh])
                    with nc.allow_low_precision(reason="bf16 attention out"):
                        nc.vector.tensor_mul(
                            outt_sb[base : base + DH, hp, qb * 512 : (qb + 1) * 512],
                            po_sbs[hh][0:DH, :],
                            zb,
                        )

            def emit_ag(blk):
                hp, qb = divmod(blk, QB)
                sl = slice(qb * 512, (qb + 1) * 512)
                nc.gpsimd.dma_start(out=cc_in[blk].ap(), in_=outt_sb[:, hp, sl])
                nc.gpsimd.collective_compute(
                    "AllGather",
                    mybir.AluOpType.bypass,
                    ins=[cc_in[blk].ap().opt()],
                    outs=[cc_out[blk].ap().opt()],
                    replica_groups=REPLICA_GROUPS,
                )
                nc.sync.dma_start(
                    out=ag_all[:, hp, qb],
                    in_=cc_out[blk].ap().rearrange("(r p) n -> p r n", p=128),
                )

            def emit_ag_half(blk, h):
                hp, qb = divmod(blk, QB)
                sl = slice(qb * 512 + h * 256, qb * 512 + (h + 1) * 256)
                nc.gpsimd.dma_start(out=cc_in_h[blk, h].ap(), in_=outt_sb[:, hp, sl])
                nc.gpsimd.collective_compute(
                    "AllGather",
                    mybir.AluOpType.bypass,
                    ins=[cc_in_h[blk, h].ap().opt()],
                    outs=[cc_out_h[blk, h].ap().opt()],
                    replica_groups=REPLICA_GROUPS,
                )
                nc.sync.dma_start(
                    out=ag_all[:, hp, qb, :, h * 256 : (h + 1) * 256],
                    in_=cc_out_h[blk, h].ap().rearrange("(r p) n -> p r n", p=128),
                )

            def emit_pass_half(blk, h):
                hp, qb = divmod(blk, QB)
                hs = slice(h * 256, (h + 1) * 256)
                for cb in range(2):
                    ps = psyp.tile([128, 512], F32, name="psy")
                    for r in range(GROUP_SIZE):
                        nc.tensor.matmul(
                            ps[:, hs],
                            lhsT=wo_sb[:, hp * 4 + r, cb * 128 : (cb + 1) * 128],
                            rhs=ag_all[:, hp, qb, r, hs],
                            start=(r == 0),
                            stop=(r == GROUP_SIZE - 1),
                        )
                    tok = slice(qb * 512 + h * 256, qb * 512 + (h + 1) * 256)
                    if hp == 0:
                        nc.vector.tensor_copy(y_acc[:, cb, tok], ps[:, hs])
                    else:
                        y_sb = yout.tile([128, 512], F32, name="y_sb")
                        nc.vector.tensor_add(y_sb[:, hs], ps[:, hs], y_acc[:, cb, tok])
                        nc.vector.tensor_scalar_add(
                            out=y_sb[:, hs], in0=y_sb[:, hs],
                            scalar1=bias_sb[:, cb : cb + 1],
                        )
                        nc.sync.dma_start(
                            out=y[cb * 128 : (cb + 1) * 128, tok], in_=y_sb[:, hs]
                        )

            def emit_pass(blk):
                hp, qb = divmod(blk, QB)
                for cb in range(2):
                    ps = psyp.tile([128, 512], F32, name="psy")
                    for r in range(GROUP_SIZE):
                        nc.tensor.matmul(
                            ps,
                            lhsT=wo_sb[:, hp * 4 + r, cb * 128 : (cb + 1) * 128],
                            rhs=ag_all[:, hp, qb, r, :],
                            start=(r == 0),
                            stop=(r == GROUP_SIZE - 1),
                        )
                    if hp == 0:
                        nc.vector.tensor_copy(y_acc[:, cb, qb * 512 : (qb + 1) * 512], ps)
                    else:
                        y_sb = yout.tile([128, 512], F32, name="y_sb")
                        nc.vector.tensor_add(
                            y_sb, ps, y_acc[:, cb, qb * 512 : (qb + 1) * 512]
                        )
                        nc.vector.tensor_scalar_add(
                            out=y_sb, in0=y_sb, scalar1=bias_sb[:, cb : cb + 1]
                        )
                        nc.sync.dma_start(
                            out=y[
                                cb * 128 : (cb + 1) * 128,
                                qb * 512 : (qb + 1) * 512,
                            ],
                            in_=y_sb,
                        )

            # ---- prelude: four interleaved chains stream behind the DMA -----
            ps_kt = psd.tile([128, 512], F32, name="psd")
            ps_qt = psd.tile([128, 512], F32, name="psd")
            ps_v0 = psyp.tile([128, 512], F32, name="psy")
            ps_v1 = psyp.tile([128, 512], F32, name="psy")
            for c in range(KC):
                st, sp = (c == 0), (c == KC - 1)
                nc.tensor.matmul(
                    ps_kt, lhsT=wk_sb[:, c, 0:128], rhs=xt_sb[:, c, 0:512],
                    start=st, stop=sp,
                )
                nc.tensor.matmul(
                    ps_qt, lhsT=wq_sb[:, c, 0:128], rhs=xt_sb[:, c, 0:512],
                    start=st, stop=sp,
                )
                nc.tensor.matmul(
                    ps_v0[:, 0:CS], lhsT=xt_sb[:, c, 0:128], rhs=wv_sb[:, c, :],
                    start=st, stop=sp,
                )
                nc.tensor.matmul(
                    ps_v1[:, 0:CS], lhsT=xt_sb[:, c, 128:256], rhs=wv_sb[:, c, :],
                    start=st, stop=sp,
                )
            with nc.allow_low_precision(reason="bf16 attention"):
                nc.vector.tensor_copy(kt_sb[:, 0, 0:512], ps_kt)
                nc.vector.tensor_copy(qt_sb[:, 0, 0:512], ps_qt)
                nc.vector.tensor_copy(
                    vaug[:, 0, :, 0:DH],
                    ps_v0[:, 0:CS].rearrange("p (h d) -> p h d", d=DH),
                )
                nc.vector.tensor_copy(
                    vaug[:, 1, :, 0:DH],
                    ps_v1[:, 0:CS].rearrange("p (h d) -> p h d", d=DH),
                )

            # deferred projection work, injected into the pipeline
            sched = {}

            def at(step, fn, *args):
                sched.setdefault(step, []).append((fn, args))

            for t in range(2, TT):
                at(t - 2, emit_v, t, psyp)
            at(2, emit_qk, 0, 1, kt_sb, wk_sb, psyp, "psy")
            at(6, emit_qk, 0, 2, kt_sb, wk_sb, psyp, "psy")
            at(10, emit_qk, 0, 3, kt_sb, wk_sb, psyp, "psy")
            at(12, emit_qk, 0, 1, qt_sb, wq_sb, psyp, "psy")
            at(20, emit_qk, 0, 2, qt_sb, wq_sb, psyp, "psy")
            at(28, emit_qk, 0, 3, qt_sb, wq_sb, psyp, "psy")
            at(34, emit_qk, 1, 0, kt_sb, wk_sb, psyp, "psy")
            at(42, emit_qk, 1, 1, kt_sb, wk_sb, psyp, "psy")
            at(50, emit_qk, 1, 2, kt_sb, wk_sb, psyp, "psy")
            at(58, emit_qk, 1, 3, kt_sb, wk_sb, psyp, "psy")
            at(38, emit_qk, 1, 0, qt_sb, wq_sb, psyp, "psy")
            at(46, emit_qk, 1, 1, qt_sb, wq_sb, psyp, "psy")
            at(66, emit_qk, 1, 2, qt_sb, wq_sb, psyp, "psy")
            at(74, emit_qk, 1, 3, qt_sb, wq_sb, psyp, "psy")

            # ---- main software-pipelined attention loop ---------------------
            pend_attv = None
            pend_recip = None
            pend_recip2 = None
            po_cur = None
            po_prev = None
            for step in range(NBLK * NKC):
                blk, kc = divmod(step, NKC)
                if kc == 0:
                    po_prev = po_cur
                    po_cur = [
                        pso.tile([DH + 1, 512], F32, name="ps_o") for _ in range(2)
                    ]
                ex = emit_dots(blk, kc)
                if pend_attv is not None:
                    pblk, pkc, pex = pend_attv
                    emit_attv(pblk, pkc, pex, po_cur if pblk == blk else po_prev)
                    if pkc == NKC - 1:
                        pend_recip2 = (pblk, *emit_recip(po_prev))
                pend_attv = (blk, kc, ex)
                if kc == 2 and blk > 0:
                    nblk, po_sbs, zinvs = pend_recip
                    emit_norm(nblk, po_sbs, zinvs)
                if kc == 3 and blk > 0:
                    emit_ag(blk - 1)
                if kc == 15 and blk > 1:
                    emit_pass(blk - 2)
                if kc == 15 and blk == NBLK - 1:
                    emit_pass(NBLK - 2)
                if kc == 0 and blk > 0:
                    pend_recip = pend_recip2
                for fn, args in sched.pop(step, []):
                    fn(*args)

            # ---- drain (blocks 0-6 were normed/gathered in-loop) ------------
            pblk, pkc, pex = pend_attv
            emit_attv(pblk, pkc, pex, po_cur)
            po_sbs7, zinvs7 = emit_recip(po_cur)
            emit_norm(pblk, po_sbs7, zinvs7)  # block 7
            emit_ag(NBLK - 1)
            emit_pass(NBLK - 1)

    nc.compile()
    return nc


_NC_CACHE = None


def _get_nc():
    global _NC_CACHE
    if _NC_CACHE is None:
        _NC_CACHE = build_nc()
    return _NC_CACHE


def _wo_perm(w_out):
    # chunk order [AG0: r0..r3 -> w_out rows 256r..256r+128,
    #              AG1: r0..r3 -> w_out rows 256r+128..256r+256]
    blocks = [w_out[256 * r : 256 * r + 128] for r in range(4)]
    blocks += [w_out[256 * r + 128 : 256 * r + 256] for r in range(4)]
    return np.concatenate(blocks, axis=0)


def _make_in_maps(x, w_qkv, w_out, b_out):
    wop = _wo_perm(w_out)
    in_maps = []
    for c in range(CORES):
        bi = c // GROUP_SIZE
        g = c % GROUP_SIZE
        cols = slice(g * CS, (g + 1) * CS)
        in_maps.append(
            {
                "xt": np.ascontiguousarray(x[bi].T).astype(NP_BF16),
                "wq": np.ascontiguousarray(w_qkv[:, cols]).astype(NP_BF16),
                "wk": np.ascontiguousarray(w_qkv[:, INNER:][:, cols]).astype(NP_BF16),
                "wv": np.ascontiguousarray(w_qkv[:, 2 * INNER:][:, cols]).astype(
                    NP_BF16
                ),
                "wo": np.ascontiguousarray(wop[:, cols]).astype(NP_BF16),
                "bo": np.ascontiguousarray(b_out[cols]),
            }
        )
    return in_maps


def _assemble(results):
    out = np.empty((B, N, DIM), dtype=np.float32)
    for c in range(CORES):
        bi = c // GROUP_SIZE
        g = c % GROUP_SIZE
        out[bi, :, g * CS : (g + 1) * CS] = results[c]["y"].T
    return out


def kernel(x, w_qkv, w_out, b_out, _trace=False, _trace_kwargs=None):
    x = np.asarray(x, dtype=np.float32)
    w_qkv = np.asarray(w_qkv, dtype=np.float32)
    w_out = np.asarray(w_out, dtype=np.float32)
    b_out = np.asarray(b_out, dtype=np.float32)
    nc = _get_nc()
    in_maps = _make_in_maps(x, w_qkv, w_out, b_out)
    res = run_bass_kernel_spmd(
        nc,
        in_maps,
        core_ids=list(range(CORES)),
        trace=_trace,
        **(_trace_kwargs or {}),
    )
    out = _assemble(res.results)
    if _trace:
        return out, res
    return out
